# revision 17
# baseline (speedup 1.0000x reference)
"""Betti-matching surrogate loss kernel for Trainium2 (8 NeuronCores).

Computes mean((probs - one_hot(gt_mask))^2) where gt_mask values are
{0,1,2} with ignore_index 2 mapped to class 0 (so class = (gt_mask == 1)).

Sharding: core k = (b, g) with b = k // 4, g = k % 4 owns
probs[b, :, 8g:8g+8, :, :] and gt_mask[b, 8g:8g+8, :, :] — contiguous
zero-copy views of the full inputs. Each core computes per-partition
partial sums of squared error; the host reduces in float64.
"""

import os

import numpy as np

import concourse.bass as bass
import concourse.mybir as mybir
from concourse.bass_utils import run_bass_kernel_spmd
from concourse.tile import TileContext


import bass_rust


def split_multiwait_instructions(nc):
    """The walrus build in this image rejects any instruction carrying more
    than one sync wait ("Too many sync wait commands"). Tile's semaphore
    assignment freely attaches several. Hoist all but the last wait of each
    instruction onto injected same-engine NoOps placed directly before it —
    engine streams execute in order, so the waits still all complete before
    the real instruction issues."""
    k = 0
    for f in nc.m.functions:
        for bb in f.blocks:
            insts = bb.instructions
            out, changed = [], False
            for inst in insts:
                si = inst.sync_info
                if si is not None and si.on_wait and len(si.on_wait) > 1:
                    SI = type(si)
                    waits = list(si.on_wait)
                    for w in waits[:-1]:
                        nop = bass_rust.InstNoOp(
                            name=f"waitsplit-{k}",
                            engine=inst.engine,
                            sync_info=SI(on_wait=[w], on_update=[]),
                        )
                        k += 1
                        nc.register_instruction(nop)
                        out.append(nop)
                    inst.sync_info = SI(
                        on_wait=[waits[-1]], on_update=list(si.on_update)
                    )
                    changed = True
                out.append(inst)
            if changed:
                bb.instructions = out

N_CORES = 8
B, C, D, H, W = 2, 2, 32, 512, 512
GROUPS = N_CORES // B          # 4 z-groups per batch
DG = D // GROUPS               # 8 z-slices per core
P = 128                        # SBUF partitions
NT = 2048                      # free-dim elements per tile
TILES = DG * H * W // (P * NT) # 8 tiles of [128, 2048] per (core, class)

_nc_cache = {}
last_results = None


def build_nc(tiles=TILES, nt=NT):
    """Per-core SPMD program: partial sum of squared error for one shard."""
    f32, i32 = mybir.dt.float32, mybir.dt.int32
    alu = mybir.AluOpType

    # Work list: (tile_idx, col_offset, width). The last tile group is
    # sub-tiled so the post-last-DMA pipeline drain is short.
    sub = 4
    chunks = [(j, 0, nt) for j in range(tiles - 1)]
    chunks += [(tiles - 1, o, nt // sub) for o in range(0, nt, nt // sub)]
    ncols = 2 * len(chunks)

    nc = bass.Bass()
    p0 = nc.dram_tensor("p0", [tiles, P, nt], f32, kind="ExternalInput")
    p1 = nc.dram_tensor("p1", [tiles, P, nt], f32, kind="ExternalInput")
    m = nc.dram_tensor("m", [tiles, P, nt], i32, kind="ExternalInput")
    out = nc.dram_tensor("out", [P, ncols], f32, kind="ExternalOutput")

    # Per-class squared error, all probs/mask tiles touched only by DVE
    # (one-column touch absorbs each DMA wait onto the DVE timeline), d
    # tiles flow DVE -> ACT, sq/acc stay on ACT. The split_multiwait pass
    # is a backstop for any residual multi-wait instruction.
    with TileContext(nc) as tc:
        with (
            tc.tile_pool(name="acc", bufs=1) as acc_pool,
            tc.tile_pool(name="mp", bufs=2) as m_pool,
            tc.tile_pool(name="tp", bufs=2) as t_pool,
            tc.tile_pool(name="pp", bufs=4) as p_pool,
            tc.tile_pool(name="dp", bufs=2) as d_pool,
            tc.tile_pool(name="sq", bufs=1) as sq_pool,
        ):
            acc = acc_pool.tile([P, ncols], f32)
            col = 0
            half_cols = 2 * (len(chunks) // 2)
            for j, off, w in chunks:
                mt = m_pool.tile([P, w], i32, tag="mt")
                nc.sync.dma_start(mt[:], m[j, :, off : off + w])
                # t = 1.0 where mask == 1 (class 1), else 0.0
                tt = t_pool.tile([P, w], f32, tag="tt")
                nc.vector.tensor_scalar(tt[:], mt[:], 1, None, op0=alu.is_equal)
                for ci, pd in ((0, p0), (1, p1)):
                    pt = p_pool.tile([P, w], f32, tag="pt")
                    nc.sync.dma_start(pt[:], pd[j, :, off : off + w])
                    # 1-column in-place touch: absorbs the DMA wait on the
                    # DVE timeline so the TT below waits on DVE alone.
                    nc.vector.tensor_scalar_mul(pt[:, :1], pt[:, :1], 1.0)
                    # c=0: d = p0 + t, err0^2 = Square(1 - d) = Square(d - 1)
                    # c=1: d = p1 - t, err1^2 = Square(d)
                    op = alu.add if ci == 0 else alu.subtract
                    d = d_pool.tile([P, w], f32, tag="d")
                    nc.vector.tensor_tensor(d[:], pt[:], tt[:], op=op)
                    # acc[:, col] = sum_free(err^2)
                    sq = sq_pool.tile([P, w], f32, tag="sq")
                    nc.scalar.activation(
                        sq[:],
                        d[:],
                        mybir.ActivationFunctionType.Square,
                        bias=1.0 if ci == 0 else 0.0,
                        scale=-1.0 if ci == 0 else 1.0,
                        accum_out=acc[:, col : col + 1],
                    )
                    col += 1
                    if col == half_cols:
                        # first half of the accumulator is final; ship early
                        nc.sync.dma_start(
                            out[:, :half_cols], acc[:, :half_cols]
                        )
            nc.sync.dma_start(out[:, half_cols:], acc[:, half_cols:])
    split_multiwait_instructions(nc)
    nc.finalize()
    return nc


def _get_nc():
    if "nc" not in _nc_cache:
        _nc_cache["nc"] = build_nc()
    return _nc_cache["nc"]


def shard_inputs(probs, gt_mask):
    in_maps = []
    for k in range(N_CORES):
        b, g = divmod(k, GROUPS)
        z0 = g * DG
        in_maps.append(
            {
                "p0": probs[b, 0, z0 : z0 + DG].reshape(TILES, P, NT),
                "p1": probs[b, 1, z0 : z0 + DG].reshape(TILES, P, NT),
                "m": gt_mask[b, z0 : z0 + DG].reshape(TILES, P, NT),
            }
        )
    return in_maps


def kernel(probs, gt_mask):
    global last_results
    probs = np.ascontiguousarray(probs, dtype=np.float32)
    gt_mask = np.ascontiguousarray(gt_mask, dtype=np.int32)
    assert probs.shape == (B, C, D, H, W) and gt_mask.shape == (B, D, H, W)

    nc = _get_nc()
    in_maps = shard_inputs(probs, gt_mask)
    trace = bool(os.environ.get("BETTI_TRACE"))
    last_results = run_bass_kernel_spmd(
        nc, in_maps, core_ids=list(range(N_CORES)), trace=trace
    )
    total = 0.0
    for r in last_results.results:
        total += r["out"].astype(np.float64).sum()
    return np.asarray(total / (B * C * D * H * W), dtype=np.float32)


# revision 18
# speedup vs baseline: 1.0966x; 1.0966x over previous
"""Betti-matching surrogate loss kernel for Trainium2 (8 NeuronCores).

Computes mean((probs - one_hot(gt_mask))^2) where gt_mask values are
{0,1,2} with ignore_index 2 mapped to class 0 (so class = (gt_mask == 1)).

Sharding: core k = (b, g) with b = k // 4, g = k % 4 owns
probs[b, :, 8g:8g+8, :, :] and gt_mask[b, 8g:8g+8, :, :] — contiguous
zero-copy views of the full inputs. Each core computes per-partition
partial sums of squared error; the host reduces in float64.
"""

import os

import numpy as np

import concourse.bass as bass
import concourse.mybir as mybir
from concourse.bass_utils import run_bass_kernel_spmd
from concourse.tile import TileContext


import bass_rust


def split_multiwait_instructions(nc):
    """The walrus build in this image rejects any instruction carrying more
    than one sync wait ("Too many sync wait commands"). Tile's semaphore
    assignment freely attaches several. Hoist all but the last wait of each
    instruction onto injected same-engine NoOps placed directly before it —
    engine streams execute in order, so the waits still all complete before
    the real instruction issues."""
    k = 0
    for f in nc.m.functions:
        for bb in f.blocks:
            insts = bb.instructions
            out, changed = [], False
            for inst in insts:
                si = inst.sync_info
                if si is not None and si.on_wait and len(si.on_wait) > 1:
                    SI = type(si)
                    waits = list(si.on_wait)
                    for w in waits[:-1]:
                        nop = bass_rust.InstNoOp(
                            name=f"waitsplit-{k}",
                            engine=inst.engine,
                            sync_info=SI(on_wait=[w], on_update=[]),
                        )
                        k += 1
                        nc.register_instruction(nop)
                        out.append(nop)
                    inst.sync_info = SI(
                        on_wait=[waits[-1]], on_update=list(si.on_update)
                    )
                    changed = True
                out.append(inst)
            if changed:
                bb.instructions = out

N_CORES = 8
B, C, D, H, W = 2, 2, 32, 512, 512
GROUPS = N_CORES // B          # 4 z-groups per batch
DG = D // GROUPS               # 8 z-slices per core
P = 128                        # SBUF partitions
NT = 2048                      # free-dim elements per tile
TILES = DG * H * W // (P * NT) # 8 tiles of [128, 2048] per (core, class)

_nc_cache = {}
last_results = None


def build_nc(tiles=TILES, nt=NT):
    """Per-core SPMD program: partial sum of squared error for one shard."""
    f32, i32 = mybir.dt.float32, mybir.dt.int32
    alu = mybir.AluOpType

    # Work list: (tile_idx, col_offset, width). The last tile group is
    # sub-tiled so the post-last-DMA pipeline drain is short.
    sub = 4
    chunks = [(j, 0, nt) for j in range(tiles - 1)]
    chunks += [(tiles - 1, o, nt // sub) for o in range(0, nt, nt // sub)]
    ncols = 2 * len(chunks)

    nc = bass.Bass()
    p0 = nc.dram_tensor("p0", [tiles, P, nt], f32, kind="ExternalInput")
    p1 = nc.dram_tensor("p1", [tiles, P, nt], f32, kind="ExternalInput")
    m = nc.dram_tensor("m", [tiles, P, nt], i32, kind="ExternalInput")
    out = nc.dram_tensor("out", [P, ncols], f32, kind="ExternalOutput")

    # Per-class squared error, all probs/mask tiles touched only by DVE
    # (one-column touch absorbs each DMA wait onto the DVE timeline), d
    # tiles flow DVE -> ACT, sq/acc stay on ACT. The split_multiwait pass
    # is a backstop for any residual multi-wait instruction.
    with TileContext(nc) as tc:
        with (
            tc.tile_pool(name="acc", bufs=1) as acc_pool,
            tc.tile_pool(name="mp", bufs=2) as m_pool,
            tc.tile_pool(name="tp", bufs=2) as t_pool,
            tc.tile_pool(name="pp", bufs=4) as p_pool,
            tc.tile_pool(name="dp", bufs=2) as d_pool,
            tc.tile_pool(name="sq", bufs=1) as sq_pool,
        ):
            acc = acc_pool.tile([P, ncols], f32)
            col = 0
            half_cols = 2 * (len(chunks) // 2)
            for j, off, w in chunks:
                mt = m_pool.tile([P, w], i32, tag="mt")
                nc.sync.dma_start(mt[:], m[j, :, off : off + w])
                # t = 1.0 where mask == 1 (class 1), else 0.0
                tt = t_pool.tile([P, w], f32, tag="tt")
                nc.vector.tensor_scalar(tt[:], mt[:], 1, None, op0=alu.is_equal)
                for ci, pd in ((0, p0), (1, p1)):
                    pt = p_pool.tile([P, w], f32, tag="pt")
                    nc.sync.dma_start(pt[:], pd[j, :, off : off + w])
                    # 1-column in-place touch: absorbs the DMA wait on the
                    # DVE timeline so the TT below waits on DVE alone.
                    nc.vector.tensor_scalar_mul(pt[:, :1], pt[:, :1], 1.0)
                    # c=0: d = p0 + t, err0^2 = Square(1 - d) = Square(d - 1)
                    # c=1: d = p1 - t, err1^2 = Square(d)
                    op = alu.add if ci == 0 else alu.subtract
                    d = d_pool.tile([P, w], f32, tag="d")
                    nc.vector.tensor_tensor(d[:], pt[:], tt[:], op=op)
                    # acc[:, col] = sum_free(err^2)
                    sq = sq_pool.tile([P, w], f32, tag="sq")
                    nc.scalar.activation(
                        sq[:],
                        d[:],
                        mybir.ActivationFunctionType.Square,
                        bias=1.0 if ci == 0 else 0.0,
                        scale=-1.0 if ci == 0 else 1.0,
                        accum_out=acc[:, col : col + 1],
                    )
                    col += 1
            nc.sync.dma_start(out[:, :], acc[:])
    split_multiwait_instructions(nc)
    nc.finalize()
    return nc


def _get_nc():
    if "nc" not in _nc_cache:
        _nc_cache["nc"] = build_nc()
    return _nc_cache["nc"]


def shard_inputs(probs, gt_mask):
    in_maps = []
    for k in range(N_CORES):
        b, g = divmod(k, GROUPS)
        z0 = g * DG
        in_maps.append(
            {
                "p0": probs[b, 0, z0 : z0 + DG].reshape(TILES, P, NT),
                "p1": probs[b, 1, z0 : z0 + DG].reshape(TILES, P, NT),
                "m": gt_mask[b, z0 : z0 + DG].reshape(TILES, P, NT),
            }
        )
    return in_maps


def kernel(probs, gt_mask):
    global last_results
    probs = np.ascontiguousarray(probs, dtype=np.float32)
    gt_mask = np.ascontiguousarray(gt_mask, dtype=np.int32)
    assert probs.shape == (B, C, D, H, W) and gt_mask.shape == (B, D, H, W)

    nc = _get_nc()
    in_maps = shard_inputs(probs, gt_mask)
    trace = bool(os.environ.get("BETTI_TRACE"))
    last_results = run_bass_kernel_spmd(
        nc, in_maps, core_ids=list(range(N_CORES)), trace=trace
    )
    total = 0.0
    for r in last_results.results:
        total += r["out"].astype(np.float64).sum()
    return np.asarray(total / (B * C * D * H * W), dtype=np.float32)
